# revision 18
# baseline (speedup 1.0000x reference)
"""Trainium2 Bass kernel for the Backflow module.

Math (B=16, N=512, DIM=3, H=32):
  out[b,i,:] = sum_j eta(||x_bi - x_bj||) * (x_bi - x_bj)  +  mu(||x_bi||) * x_bi
where eta/mu are 1->H->1 tanh MLPs. The reference's eye()/diagonal correction
cancels exactly (eta(d_ii) multiplies r_ii = 0).

Host-side refit: eta is smooth and univariate, so the host refits it as a
3-unit tanh network *in squared-distance space*:
  eta(d) ~ sum_{k<3} w2_k tanh(w1_k u + b1_k) + b2,   u = d^2
(weighted LS on a grid; w2 returned fp16-exact via greedy quantize-and-
resolve — large canceling w2 pairs are poison: their fp16 rounding error is
amplified ~50x by the 512-term row sums). u-space removes every Sqrt from
the device program. mu is per-particle (only B*N = 8K evals), so the host
computes m_i = mu(||x_i||) exactly and ships it; no mu fit, no mu columns.

Per-core (2 batches/core on 8 cores, tiny tensors replicated):
  d^2 strips on the PE in plain fp16 (measured end-to-end cost of fp16 vs
  f32r here: +5e-6 rel err; d^2 errors are per-element and wash out in the
  row sums): stationary [-2x_i | 1 | n_i] x moving [x_j | n_j | 1] -> PSUM.
  i on partitions (4 chunks of 128), j on the free dim, block-triangular
  strips packed [128, 1280] in strip order [0,1,3,2] so every matmul dst /
  cast region stays inside a PSUM bank. PSUM->SBUF fp16 casts alternate
  DVE/GpSimd so neither engine serializes the chain.

  G accumulated in PSUM by 3 tanh ACT passes x diag(w2_k) fp16 matmuls;
  the k0 ACT pass is split per-bank so each G matmul starts as soon as its
  columns are cast. b2 is folded into G during the PSUM->SBUF staging
  (tensor_scalar add / ACT Identity+bias), which kills the old
  ident-stationary bias matmul: with G' = G + b2,
    out[i,c] = x[i,c]*(Q'_i + m_i) - P'_c   exactly.

  Contractions use G' blocks as PE stationary with the 6-wide moving
  [x_c | 1] -> per-row-chunk [128, 6] PSUM (P' cols 0:3, Q' 3:6). Rows run
  0,1,2,3 with each row's reflected-block transposes emitted just before
  its contribs, so the LAST row (3) is transpose-free and the tail chain is
  short. Output DMA'd in row pairs ({0,1} then {2,3}) on alternating queues.

Latency engineering (the big one): the PE clock governor (HAM) only
un-throttles 1.2->2.4 GHz after a ~3.4us window of *gapless* PE activity —
the old 6-matmul warmup (2.7us) never tripped it and the whole kernel ran
at 1.2 GHz (confirmed from ntff ham events: K=8 only for 25.6->29.0us of a
35us run). 8 back-to-back 512-col warmup matmuls now fill the input-DMA
window and trip the governor right as the real work starts. Input DMAs are
merged into 3 descriptors (xTn|statd fp16; per-partition smalls; statx) —
the old w2d (96KB) / ident (32KB) operands are built on-device with
gpsimd affine_selects instead of DMA'd.
"""

import sys

sys.path.insert(0, "/opt/trn_rl_repo")

import numpy as np
from contextlib import ExitStack

B, N, DIM, H = 16, 512, 3, 32
HP = 2  # refitted tanh eta units (u-space); unit 2 is linear in u (free)
NCORES = 8
BPC = B // NCORES  # batches per core
P = 128
NCHUNK = N // P  # 4
NROW = DIM + 2  # d^2 matmul contraction rows: x(3), n, ones
# block-triangular strips: chunk I covers j in [128*I, N); packed in order
# [0,1,3,2] so strip boundaries align with PSUM banks (512 f32 cols)
SORDER = [0, 1, 3, 2]
WIDTHS = {I: N - P * I for I in range(NCHUNK)}  # 512, 384, 256, 128
OFFS = {}
_off = 0
for _I in SORDER:
    OFFS[_I] = _off
    _off += WIDTHS[_I]
NPACK = _off  # 1280
# PSUM-bank-sized column splits of the packed strip for the diag matmuls
MM_SPLITS = [(0, 512), (512, 512), (1024, 256)]
N_WARMUP = 8  # 512-col dummy PE matmuls: >=3.4us gapless to trip the HAM
# smalls column layout (one [P, NS] f32 tensor, values replicated per row)
SC_W2 = 0  # HP + 1 cols: w2_0, w2_1, a (linear-unit slope)
SC_W1 = SC_W2 + HP + 1
SC_B1 = SC_W1 + HP
SC_BS = SC_B1 + HP  # BPC*DIM cols of b2*S (per-batch column sums)
SC_M = SC_BS + BPC * DIM  # BPC*NCHUNK cols of host mu (+ b2*N folded in)
NS = SC_M + BPC * NCHUNK

LAST_RESULT = None
_PROGRAM_CACHE = {}


def _spread_sync_waits(nc):
    """The pinned walrus rejects instructions carrying more than one sync wait
    ('Too many sync wait commands'). Engines execute their instruction streams
    in order, so hoist all-but-one wait of any such instruction onto same-engine
    NoOps inserted directly before it — semantically identical ordering."""
    from concourse import mybir

    n_added = 0
    for bb in nc.main_func.blocks:
        insts = bb.instructions
        i = 0
        while i < len(insts):
            inst = insts[i]
            si = getattr(inst, "sync_info", None)
            waits = list(si.on_wait) if si is not None and si.on_wait else []
            if len(waits) > 1:
                si.on_wait = waits[-1:]
                for k, w in enumerate(waits[:-1]):
                    nop = mybir.InstNoOp(
                        name=f"{inst.name}-wspread{k}",
                        sync_info=mybir.SyncInfo(on_wait=[w], on_update=[]),
                        engine=inst.engine,
                        bass_nofuse=True,
                    )
                    insts.insert(i + k, nop)
                    n_added += 1
                i += len(waits) - 1
            i += 1
    return n_added


def _build_program():
    import concourse.bass as bass
    import concourse.tile as tile
    from concourse import mybir

    f32 = mybir.dt.float32
    f16 = mybir.dt.float16
    AF = mybir.ActivationFunctionType
    OP = mybir.AluOpType

    nc = bass.Bass()
    dm_d = nc.dram_tensor("dm", [NROW, BPC, N + NCHUNK * P], f16, kind="ExternalInput")
    smalls_d = nc.dram_tensor("smalls", [P, NS], f32, kind="ExternalInput")
    statx_d = nc.dram_tensor("statx", [P, BPC, NCHUNK, 2 * DIM], f16, kind="ExternalInput")
    out_d = nc.dram_tensor("out", [BPC, P, NCHUNK, DIM], f32, kind="ExternalOutput")

    with tile.TileContext(nc) as tc, ExitStack() as ctx:
        singles = ctx.enter_context(tc.tile_pool(name="singles", bufs=1))
        dqp = ctx.enter_context(tc.tile_pool(name="dqp", bufs=1))
        hp0 = ctx.enter_context(tc.tile_pool(name="hp0", bufs=3))
        accsbp = ctx.enter_context(tc.tile_pool(name="accsbp", bufs=2))
        atp = ctx.enter_context(tc.tile_pool(name="atp", bufs=8))
        enp = ctx.enter_context(tc.tile_pool(name="enp", bufs=2))
        orp = ctx.enter_context(tc.tile_pool(name="orp", bufs=2))
        psacc = ctx.enter_context(tc.tile_pool(name="psacc", bufs=1, space="PSUM"))
        psout = ctx.enter_context(tc.tile_pool(name="psout", bufs=1, space="PSUM"))
        psd2 = ctx.enter_context(tc.tile_pool(name="psd2", bufs=3, space="PSUM"))

        # ---- PE warmup: >=3.4us of gapless data-independent matmuls during
        # the input-DMA window trip the HAM clock governor (1.2 -> 2.4 GHz)
        # right as the real work starts.
        wu_sb = singles.tile([P, 512], f16)
        nc.vector.memset(wu_sb[:], 0.25)
        for _ in range(N_WARMUP):
            wt = psd2.tile([P, 512], f32, tag="d2")
            nc.tensor.matmul(wt[:], wu_sb[:, 0:P], wu_sb[:], start=True, stop=True)
        # dummy 1-col tanh: pulls the 1.3us ACT_TABLE_LOAD into the DMA
        # window instead of serializing it before the first real k0 pass
        tanh_wu = singles.tile([P, 1], f16)
        nc.scalar.activation(tanh_wu[:], wu_sb[:, 0:1], AF.Tanh, scale=1.0, bias=0.0)

        # ---- input DMAs: 3 merged descriptors on two queues
        dm_sb = singles.tile([NROW, BPC, N + NCHUNK * P], f16)
        nc.sync.dma_start(out=dm_sb[:], in_=dm_d[:])
        smalls_sb = singles.tile([P, NS], f32)
        nc.sync.dma_start(out=smalls_sb[:], in_=smalls_d[:])
        statx_sb = singles.tile([P, BPC, NCHUNK, 2 * DIM], f16)
        nc.sync.dma_start(out=statx_sb[:], in_=statx_d[:])

        def m_ap(b, R):
            c = SC_M + b * NCHUNK + R
            return smalls_sb[:, c : c + 1]

        # ---- on-device operand builds (gpsimd), replacing 128KB of DMA:
        # ident for the PE transposes; w2d = stacked diag(w2_k) fp16.
        ident_sb = singles.tile([P, P], f16)
        nc.gpsimd.memset(ident_sb[:], 1.0)
        nc.gpsimd.affine_select(
            out=ident_sb[:],
            in_=ident_sb[:],
            compare_op=OP.is_equal,
            fill=0.0,
            base=0,
            pattern=[[-1, P]],
            channel_multiplier=1,
        )
        w2v16 = singles.tile([P, HP + 1], f16)
        nc.gpsimd.tensor_copy(w2v16[:], smalls_sb[:, SC_W2 : SC_W2 + HP + 1])
        w2d_sb = singles.tile([P, HP + 1, P], f16)
        for k in range(HP + 1):
            nc.gpsimd.affine_select(
                out=w2d_sb[:, k, :],
                in_=w2v16[:, k : k + 1].to_broadcast([P, P]),
                compare_op=OP.is_equal,
                fill=0.0,
                base=0,
                pattern=[[-1, P]],
                channel_multiplier=1,
            )
        # bs6: [b2*S_c | 0 0 0] per batch, the moving operand of the per-row
        # ident-stationary matmul that folds the b2*S correction into PSUM
        bs6_sb = singles.tile([P, BPC, 2 * DIM], f16)
        nc.gpsimd.memset(bs6_sb[:], 0.0)
        for bb in range(BPC):
            nc.gpsimd.tensor_copy(
                bs6_sb[:, bb, 0:DIM], smalls_sb[:, SC_BS + bb * DIM : SC_BS + (bb + 1) * DIM]
            )

        # ---- per-(batch, chunk) d^2 matmul + fp16 staging cast ----
        # the k0 tanh ACT reads the d^2 PSUM *directly*, so the DVE cast
        # (which only feeds k1/k2) is off the critical path entirely
        ds_all = {
            b: dqp.tile([P, NPACK], f16, tag=f"ds{b}", name=f"ds{b}")
            for b in range(BPC)
        }
        d2ps_all = {}

        def prep(b, I):
            if I == 2:
                d2ps = psout.tile([P, WIDTHS[I]], f32, tag=("pqa", "pqb")[b])
            else:
                d2ps = psd2.tile([P, WIDTHS[I]], f32, tag="d2")
            d2ps_all[(b, I)] = d2ps
            nc.tensor.matmul(
                d2ps[:],
                dm_sb[:, b, N + I * P : N + (I + 1) * P],
                dm_sb[:, b, P * I : N],
                start=True,
                stop=True,
            )

        def make_reflection(b, acc_sb):
            """Closures for the contractions (G' blocks as PE stationary,
            [x|1] 6-wide moving -> per-row [128, 6] PSUM, P' in cols 0:3,
            Q' in 3:6), JIT transposes for the reflected blocks, per-row
            finalize, and row-pair output DMAs. Row order 0,1,2,3 so the
            last row needs no transposes (short tail)."""

            def blk(I, J):
                off = OFFS[I] + (J - I) * P
                return acc_sb[:, off : off + P]

            # adjacent rows alternate PSUM tiles (banks) so a row's
            # accumulation never serializes against the previous row's
            # finalize reads
            pqa = psout.tile([P, 2, 2 * DIM], f32, tag="pqa")
            pqb = psout.tile([P, 2, 2 * DIM], f32, tag="pqb")

            def pq_slot(row):
                return (pqa, pqb)[row % 2], row // 2

            nfirst = {id(pqa): True, id(pqb): True}

            def contrib(row, stat_chunk, stationary):
                t, r = pq_slot(row)
                nc.tensor.matmul(
                    t[:, r, :],
                    stationary,
                    statx_sb[:, b, stat_chunk, :],
                    start=nfirst[id(t)],
                    stop=False,
                    skip_group_check=True,
                )
                nfirst[id(t)] = False

            def bs_add(row):
                # += [b2*S_c | 0] via ident-stationary matmul, closing the
                # row's accumulation group
                t, r = pq_slot(row)
                nc.tensor.matmul(
                    t[:, r, :],
                    ident_sb[:],
                    bs6_sb[:, b, :],
                    start=False,
                    stop=True,
                    skip_group_check=True,
                )

            at_tiles = {}

            def trans_only(I, J):
                tps = psd2.tile([P, P], f16, tag="d2")
                nc.tensor.transpose(tps[:], blk(I, J), ident_sb[:])
                at_sb = atp.tile([P, P], f16)
                nc.vector.tensor_copy(at_sb[:], tps[:])
                at_tiles[(I, J)] = at_sb

            outrow = orp.tile([P, NCHUNK, DIM], f32)

            def fin_row(R):
                # out[i,c] = x[i,c]*(Q'_i + m_i) - P'[i,c]
                pt, r = pq_slot(R)
                t = enp.tile([P, DIM], f32, tag="t")
                nc.vector.scalar_tensor_tensor(
                    out=t[:],
                    in0=pt[:, r, DIM : 2 * DIM],
                    scalar=m_ap(b, R),
                    in1=statx_sb[:, b, R, 0:DIM],
                    op0=OP.add,
                    op1=OP.mult,
                )
                nc.vector.tensor_sub(outrow[:, R, :], t[:], pt[:, r, 0:DIM])

            ops = []
            for row in range(NCHUNK):
                for J in range(row + 1, NCHUNK):
                    ops.append(lambda row=row, J=J: trans_only(row, J))
                # contrib arg lists: diag, direct (I<row), reflected (J>row);
                # the final one carries stop=True for its PSUM region
                cargs = [(row, lambda row=row: blk(row, row))]
                for I in range(row):
                    cargs.append((I, lambda row=row, I=I: blk(I, row)))
                for J in range(row + 1, NCHUNK):
                    cargs.append((J, lambda row=row, J=J: at_tiles[(row, J)][:]))
                for chunk, statf in cargs:
                    ops.append(
                        lambda row=row, chunk=chunk, statf=statf: contrib(
                            row, chunk, statf()
                        )
                    )
                ops.append(lambda row=row: bs_add(row))
                ops.append(lambda row=row: fin_row(row))
                if row == 1:
                    ops.append(
                        lambda: nc.gpsimd.dma_start(
                            out=out_d[b][:, 0:2, :], in_=outrow[:, 0:2, :]
                        )
                    )
                if row == NCHUNK - 1:
                    ops.append(
                        lambda: nc.sync.dma_start(
                            out=out_d[b][:, 2:NCHUNK, :], in_=outrow[:, 2:NCHUNK, :]
                        )
                    )
            return ops

        # ---- main per-batch flow ----
        pending = []
        for b in range(BPC):
            for I in SORDER:
                prep(b, I)
        for b in range(BPC):
            ds = ds_all[b]
            acc = psacc.tile([P, NPACK], f32)
            acc_sb = accsbp.tile([P, NPACK], f16)
            hs_k = {}
            for k in range(HP):
                hs = hp0.tile([P, NPACK], f16, tag="hs")
                hs_k[k] = hs
                scale = smalls_sb[:, SC_W1 + k : SC_W1 + k + 1]
                bias = smalls_sb[:, SC_B1 + k : SC_B1 + k + 1]
                if k == 0 and b == 0:
                    # c2 cast first: ACT k0-s2 is reached late anyway, and
                    # this ordering keeps c2 off the DVE tail so the k1
                    # pass (which reads all of ds) starts on time
                    nc.vector.tensor_copy(
                        ds[:, OFFS[2] : OFFS[2] + WIDTHS[2]], d2ps_all[(b, 2)][:]
                    )
                    # batch 0 k0 reads the d^2 PSUM directly per strip
                    for I in SORDER:
                        nc.scalar.activation(
                            hs[:, OFFS[I] : OFFS[I] + WIDTHS[I]],
                            d2ps_all[(b, I)][:],
                            AF.Tanh,
                            scale=scale,
                            bias=bias,
                        )
                    # remaining casts: b0 (feeds b0 k1 + linear), then b1
                    for I in (0, 1, 3):
                        nc.vector.tensor_copy(
                            ds[:, OFFS[I] : OFFS[I] + WIDTHS[I]], d2ps_all[(b, I)][:]
                        )
                    for I in SORDER:
                        nc.vector.tensor_copy(
                            ds_all[1][:, OFFS[I] : OFFS[I] + WIDTHS[I]],
                            d2ps_all[(1, I)][:],
                        )
                else:
                    nc.scalar.activation(
                        hs[:, 0:NPACK],
                        ds[:, 0:NPACK],
                        AF.Tanh,
                        scale=scale,
                        bias=bias,
                    )
                for off, w in MM_SPLITS:
                    nc.tensor.matmul(
                        acc[:, off : off + w],
                        w2d_sb[:, k, :],
                        hs[:, off : off + w],
                        start=(k == 0),
                        stop=False,
                    )
                if b == 0 and k == 0:
                    # gapless dummy matmuls in the pqa bank keep the HAM
                    # busy-window full while the PE waits for the k1 ACT
                    for _ in range(18):
                        dt = psout.tile([P, 256], f32, tag="pqa")
                        nc.tensor.matmul(
                            dt[:], wu_sb[:, 0:P], wu_sb[:, 0:256],
                            start=True, stop=True,
                        )
                if b == 1 and k == 0:
                    # early transposes of batch 0 soak the PE gap before
                    # this batch's k1 matmuls
                    for _ in range(3):
                        if pending:
                            pending.pop(0)()
            # linear unit: diag(a) x ds -> acc, no ACT pass at all.
            # All three matmuls BEFORE any staging read: a staging read of
            # the acc slot WAR-blocks every later matmul into it (PSUM dep
            # tracking is slot-granular), so interleaving would serialize
            # a PE<->DVE ping-pong chain
            for off, w in MM_SPLITS:
                nc.tensor.matmul(
                    acc[:, off : off + w],
                    w2d_sb[:, HP, :],
                    ds[:, off : off + w],
                    start=False,
                    stop=True,
                )
            for off, w in MM_SPLITS:
                # plain casts (cheapest PSUM->SBUF op); the last batch
                # borrows the freshly-idle ACT for two splits
                if b == BPC - 1 and off >= 512:
                    nc.scalar.copy(acc_sb[:, off : off + w], acc[:, off : off + w])
                else:
                    nc.vector.tensor_copy(
                        acc_sb[:, off : off + w], acc[:, off : off + w]
                    )
            if b == 0:
                # more gap-filler matmuls (pqb bank) bridging to the b1-k0
                # ACT pass
                for _ in range(8):
                    dt = psout.tile([P, 256], f32, tag="pqb")
                    nc.tensor.matmul(
                        dt[:], wu_sb[:, 0:P], wu_sb[:, 0:256],
                        start=True, stop=True,
                    )
            while pending:
                pending.pop(0)()
            pending = make_reflection(b, acc_sb)
        while pending:
            pending.pop(0)()

    _spread_sync_waits(nc)
    return nc


def _fit_eta(ug, target, wts, hp=HP, seed=1, n_starts=30):
    """Weighted LS refit of eta in u = d^2 space as hp tanh units plus a
    linear term:  eta(u) ~ sum_k w2_k tanh(w1_k u + b1_k) + a u + b2.

    The linear term costs nothing on device (the staged u tile feeds a
    diag(a) matmul directly, no ACT pass), and with it 2 tanh units beat
    the old 3-unit fit. The w2/a coefficients end up as fp16 diag-matmul
    stationaries, so a final greedy pass rounds them to fp16 one at a time
    (largest effect first), re-solving the rest against the residual —
    large canceling coefficients are poison: their fp16 rounding error is
    amplified ~50x by the 512-term row sums."""
    from scipy.optimize import least_squares

    umax = max(ug.max(), 1.0)

    def lin_solve(w1, b1, lam=5e-2):
        Phi = np.concatenate(
            [np.tanh(ug[:, None] * w1 + b1), ug[:, None] / umax, np.ones((len(ug), 1))],
            1,
        )
        A = Phi * wts[:, None]
        Aaug = np.concatenate([A, lam * np.eye(Phi.shape[1])], axis=0)
        baug = np.concatenate([target * wts, np.zeros(Phi.shape[1])])
        cc, *_ = np.linalg.lstsq(Aaug, baug, rcond=None)
        return cc, Phi

    rng = np.random.default_rng(seed)
    best = None
    for _ in range(n_starts):
        w1 = rng.uniform(0.1, 1.0, hp) * rng.choice([-1, 1], hp) / umax
        b1 = -w1 * rng.uniform(0, umax, hp)

        def resid(p):
            cc, Phi = lin_solve(p[:hp], p[hp:])
            return (Phi @ cc - target) * wts

        try:
            res = least_squares(
                resid, np.concatenate([w1, b1]), method="lm", max_nfev=400
            )
        except Exception:
            continue
        if best is None or res.cost < best[0]:
            best = (res.cost, res.x)
    _, p = best
    w1o, b1o = p[:hp], p[hp:]
    cc, _ = lin_solve(w1o, b1o)

    # greedy fp16-aware rounding of [w2_0, w2_1, a] (ridge re-solves)
    Phi = np.concatenate([np.tanh(ug[:, None] * w1o + b1o), ug[:, None]], 1)
    A = Phi * wts[:, None]
    y = target * wts
    co = np.concatenate([cc[:hp], [cc[hp] / umax]])
    q = np.zeros(hp + 1)
    b2o = float(cc[hp + 1])
    free = list(range(hp + 1))
    scalev = np.concatenate([np.ones(hp), [ug.max()]])
    for _ in range(hp + 1):
        j = max(free, key=lambda i: abs(co[i] * scalev[i]))
        q[j] = float(np.float16(co[j]))
        free.remove(j)
        fixed = [i for i in range(hp + 1) if i not in free]
        y2 = y - A[:, fixed] @ q[fixed]
        Af = np.concatenate([A[:, free], wts[:, None]], 1)
        nf = Af.shape[1]
        Aaug = np.concatenate([Af, 1e-3 * np.eye(nf)], 0)
        yaug = np.concatenate([y2, np.zeros(nf)])
        cc2, *_ = np.linalg.lstsq(Aaug, yaug, rcond=None)
        if free:
            co[free] = cc2[:-1]
        b2o = float(cc2[-1])
    return w1o, b1o, q[:hp], q[hp], b2o


def _ensure_ntff_hook():
    """bass_utils' axon trace path imports antenv.axon_hooks, which the image's
    antenv package lacks. Register an equivalent module backed by the boot
    package's ctypes NTFF hook so trace=True works; degrade silently if the
    pieces are missing (tracing is optional)."""
    import os
    import types

    try:
        import antenv.axon_hooks  # noqa: F401

        return
    except ImportError:
        pass
    try:
        import antenv
    except ImportError:
        return
    mod = types.ModuleType("antenv.axon_hooks")
    box = {"h": None}
    mod.set_axon_ntff_profile_hook = lambda h: box.__setitem__("h", h)
    mod.get_axon_ntff_profile_hook = lambda: box["h"]
    sys.modules["antenv.axon_hooks"] = mod
    antenv.axon_hooks = mod
    try:
        from trn_agent_boot.trn_boot import _ntff_profile_via_ctypes

        so = "/opt/axon/libaxon_pjrt.so"
        if os.path.exists(so):
            hook = _ntff_profile_via_ctypes(so)
            if hook is not None:
                mod.set_axon_ntff_profile_hook(hook)
    except Exception:
        pass


def kernel(x, eta_w1, eta_b1, eta_w2, eta_b2, mu_w1, mu_b1, mu_w2, mu_b2):
    global LAST_RESULT
    _ensure_ntff_hook()
    from concourse.bass_utils import run_bass_kernel_spmd

    f32 = np.float32
    f16 = np.float16
    x = np.ascontiguousarray(np.asarray(x, dtype=f32))
    eta_w1 = np.asarray(eta_w1, f32)
    eta_b1 = np.asarray(eta_b1, f32)
    eta_w2 = np.asarray(eta_w2, f32)
    eta_b2 = np.asarray(eta_b2, f32)
    mu_w1 = np.asarray(mu_w1, f32)
    mu_b1 = np.asarray(mu_b1, f32)
    mu_w2 = np.asarray(mu_w2, f32)
    mu_b2 = np.asarray(mu_b2, f32)

    norms = np.linalg.norm(x.astype(np.float64), axis=2)
    dmax = 2.0 * norms.max()
    # eta refit in u = d^2 space (2 tanh units + linear term)
    dg = np.linspace(0.0, dmax, 1200)
    eta_t = np.tanh(dg[:, None] * eta_w1[0].astype(np.float64) + eta_b1) @ eta_w2[
        :, 0
    ].astype(np.float64) + float(eta_b2[0])
    w1f, b1f, w2f, af, b2f = _fit_eta(dg * dg, eta_t, np.maximum(dg, 0.05))
    # mu is per-particle: computed exactly on host, no fit
    m_host = np.tanh(norms[..., None] * mu_w1[0].astype(np.float64) + mu_b1) @ mu_w2[
        :, 0
    ].astype(np.float64) + float(mu_b2[0])  # [B, N]

    if "prog" not in _PROGRAM_CACHE:
        _PROGRAM_CACHE["prog"] = _build_program()
    nc = _PROGRAM_CACHE["prog"]

    smalls_base = np.zeros(NS, f32)
    smalls_base[SC_W2 : SC_W2 + HP] = w2f
    smalls_base[SC_W2 + HP] = af
    smalls_base[SC_W1 : SC_W1 + HP] = w1f
    smalls_base[SC_B1 : SC_B1 + HP] = b1f

    in_maps = []
    for core in range(NCORES):
        xc = np.ascontiguousarray(x[core * BPC : (core + 1) * BPC])
        xTc = xc.transpose(0, 2, 1)  # [BPC, DIM, N]
        n2 = (xc.astype(np.float64) ** 2).sum(axis=2).astype(f32)  # [BPC, N]
        dm = np.empty((NROW, BPC, N + NCHUNK * P), f16)
        # moving part: [x_j | n_j | 1]
        dm[0:DIM, :, 0:N] = xTc.transpose(1, 0, 2)
        dm[DIM, :, 0:N] = n2
        dm[DIM + 1, :, 0:N] = 1.0
        # stationary strips: [-2x_i | 1 | n_i]
        statx = np.empty((P, BPC, NCHUNK, 2 * DIM), f16)
        for bb in range(BPC):
            for I in range(NCHUNK):
                dm[0:DIM, bb, N + I * P : N + (I + 1) * P] = (
                    -2.0 * xTc[bb, :, I * P : (I + 1) * P]
                )
                dm[DIM, bb, N + I * P : N + (I + 1) * P] = 1.0
                dm[DIM + 1, bb, N + I * P : N + (I + 1) * P] = n2[
                    bb, I * P : (I + 1) * P
                ]
                statx[:, bb, I, 0:DIM] = xc[bb, I * P : (I + 1) * P].astype(f16)
                statx[:, bb, I, DIM : 2 * DIM] = 1.0
        smalls = np.tile(smalls_base, (P, 1))
        S = xc.astype(np.float64).sum(axis=1)  # [BPC, DIM]
        mloc = m_host[core * BPC : (core + 1) * BPC] + b2f * N  # [BPC, N]
        for bb in range(BPC):
            smalls[:, SC_BS + bb * DIM : SC_BS + (bb + 1) * DIM] = (b2f * S[bb])[None]
            for I in range(NCHUNK):
                smalls[:, SC_M + bb * NCHUNK + I] = mloc[bb, I * P : (I + 1) * P]
        in_maps.append(
            {
                "dm": np.ascontiguousarray(dm),
                "smalls": np.ascontiguousarray(smalls.astype(f32)),
                "statx": statx,
            }
        )

    res = run_bass_kernel_spmd(nc, in_maps, core_ids=list(range(NCORES)))
    LAST_RESULT = res
    out = np.concatenate([r["out"] for r in res.results], axis=0)  # [B, P, NCHUNK, DIM]
    out = out.transpose(0, 2, 1, 3).reshape(B, N, DIM)
    return np.ascontiguousarray(out).astype(np.float32)


# revision 19
# speedup vs baseline: 1.0571x; 1.0571x over previous
"""Trainium2 Bass kernel for the Backflow module.

Math (B=16, N=512, DIM=3, H=32):
  out[b,i,:] = sum_j eta(||x_bi - x_bj||) * (x_bi - x_bj)  +  mu(||x_bi||) * x_bi
where eta/mu are 1->H->1 tanh MLPs. The reference's eye()/diagonal correction
cancels exactly (eta(d_ii) multiplies r_ii = 0).

Host-side refit: eta is smooth and univariate, so the host refits it as a
3-unit tanh network *in squared-distance space*:
  eta(d) ~ sum_{k<3} w2_k tanh(w1_k u + b1_k) + b2,   u = d^2
(weighted LS on a grid; w2 returned fp16-exact via greedy quantize-and-
resolve — large canceling w2 pairs are poison: their fp16 rounding error is
amplified ~50x by the 512-term row sums). u-space removes every Sqrt from
the device program. mu is per-particle (only B*N = 8K evals), so the host
computes m_i = mu(||x_i||) exactly and ships it; no mu fit, no mu columns.

Per-core (2 batches/core on 8 cores, tiny tensors replicated):
  d^2 strips on the PE in plain fp16 (measured end-to-end cost of fp16 vs
  f32r here: +5e-6 rel err; d^2 errors are per-element and wash out in the
  row sums): stationary [-2x_i | 1 | n_i] x moving [x_j | n_j | 1] -> PSUM.
  i on partitions (4 chunks of 128), j on the free dim, block-triangular
  strips packed [128, 1280] in strip order [0,1,3,2] so every matmul dst /
  cast region stays inside a PSUM bank. PSUM->SBUF fp16 casts alternate
  DVE/GpSimd so neither engine serializes the chain.

  G accumulated in PSUM by 3 tanh ACT passes x diag(w2_k) fp16 matmuls;
  the k0 ACT pass is split per-bank so each G matmul starts as soon as its
  columns are cast. b2 is folded into G during the PSUM->SBUF staging
  (tensor_scalar add / ACT Identity+bias), which kills the old
  ident-stationary bias matmul: with G' = G + b2,
    out[i,c] = x[i,c]*(Q'_i + m_i) - P'_c   exactly.

  Contractions use G' blocks as PE stationary with the 6-wide moving
  [x_c | 1] -> per-row-chunk [128, 6] PSUM (P' cols 0:3, Q' 3:6). Rows run
  0,1,2,3 with each row's reflected-block transposes emitted just before
  its contribs, so the LAST row (3) is transpose-free and the tail chain is
  short. Output DMA'd in row pairs ({0,1} then {2,3}) on alternating queues.

Latency engineering (the big one): the PE clock governor (HAM) only
un-throttles 1.2->2.4 GHz after a ~3.4us window of *gapless* PE activity —
the old 6-matmul warmup (2.7us) never tripped it and the whole kernel ran
at 1.2 GHz (confirmed from ntff ham events: K=8 only for 25.6->29.0us of a
35us run). 8 back-to-back 512-col warmup matmuls now fill the input-DMA
window and trip the governor right as the real work starts. Input DMAs are
merged into 3 descriptors (xTn|statd fp16; per-partition smalls; statx) —
the old w2d (96KB) / ident (32KB) operands are built on-device with
gpsimd affine_selects instead of DMA'd.
"""

import sys

sys.path.insert(0, "/opt/trn_rl_repo")

import numpy as np
from contextlib import ExitStack

B, N, DIM, H = 16, 512, 3, 32
HP = 2  # refitted tanh eta units (u-space); unit 2 is linear in u (free)
NCORES = 8
BPC = B // NCORES  # batches per core
P = 128
NCHUNK = N // P  # 4
NROW = DIM + 2  # d^2 matmul contraction rows: x(3), n, ones
# block-triangular strips: chunk I covers j in [128*I, N); packed in order
# [0,1,3,2] so strip boundaries align with PSUM banks (512 f32 cols)
SORDER = [0, 1, 3, 2]
WIDTHS = {I: N - P * I for I in range(NCHUNK)}  # 512, 384, 256, 128
OFFS = {}
_off = 0
for _I in SORDER:
    OFFS[_I] = _off
    _off += WIDTHS[_I]
NPACK = _off  # 1280
# PSUM-bank-sized column splits of the packed strip for the diag matmuls
MM_SPLITS = [(0, 512), (512, 512), (1024, 256)]
N_WARMUP = 8  # 512-col dummy PE matmuls: >=3.4us gapless to trip the HAM
# smalls column layout (one [P, NS] f32 tensor, values replicated per row)
SC_W2 = 0  # HP + 1 cols: w2_0, w2_1, a (linear-unit slope)
SC_W1 = SC_W2 + HP + 1
SC_B1 = SC_W1 + HP
SC_BS = SC_B1 + HP  # BPC*DIM cols of b2*S (per-batch column sums)
SC_M = SC_BS + BPC * DIM  # BPC*NCHUNK cols of host mu (+ b2*N folded in)
NS = SC_M + BPC * NCHUNK

LAST_RESULT = None
_PROGRAM_CACHE = {}


def _spread_sync_waits(nc):
    """The pinned walrus rejects instructions carrying more than one sync wait
    ('Too many sync wait commands'). Engines execute their instruction streams
    in order, so hoist all-but-one wait of any such instruction onto same-engine
    NoOps inserted directly before it — semantically identical ordering."""
    from concourse import mybir

    n_added = 0
    for bb in nc.main_func.blocks:
        insts = bb.instructions
        i = 0
        while i < len(insts):
            inst = insts[i]
            si = getattr(inst, "sync_info", None)
            waits = list(si.on_wait) if si is not None and si.on_wait else []
            if len(waits) > 1:
                si.on_wait = waits[-1:]
                for k, w in enumerate(waits[:-1]):
                    nop = mybir.InstNoOp(
                        name=f"{inst.name}-wspread{k}",
                        sync_info=mybir.SyncInfo(on_wait=[w], on_update=[]),
                        engine=inst.engine,
                        bass_nofuse=True,
                    )
                    insts.insert(i + k, nop)
                    n_added += 1
                i += len(waits) - 1
            i += 1
    return n_added


def _build_program():
    import concourse.bass as bass
    import concourse.tile as tile
    from concourse import mybir

    f32 = mybir.dt.float32
    f16 = mybir.dt.float16
    AF = mybir.ActivationFunctionType
    OP = mybir.AluOpType

    nc = bass.Bass()
    dm_d = nc.dram_tensor("dm", [NROW, BPC, N + NCHUNK * P], f16, kind="ExternalInput")
    smalls_d = nc.dram_tensor("smalls", [P, NS], f32, kind="ExternalInput")
    statx_d = nc.dram_tensor("statx", [P, BPC, NCHUNK, 2 * DIM], f16, kind="ExternalInput")
    out_d = nc.dram_tensor("out", [BPC, P, NCHUNK, DIM], f32, kind="ExternalOutput")

    with tile.TileContext(nc) as tc, ExitStack() as ctx:
        singles = ctx.enter_context(tc.tile_pool(name="singles", bufs=1))
        dqp = ctx.enter_context(tc.tile_pool(name="dqp", bufs=1))
        hp0 = ctx.enter_context(tc.tile_pool(name="hp0", bufs=3))
        accsbp = ctx.enter_context(tc.tile_pool(name="accsbp", bufs=2))
        atp = ctx.enter_context(tc.tile_pool(name="atp", bufs=8))
        enp = ctx.enter_context(tc.tile_pool(name="enp", bufs=2))
        orp = ctx.enter_context(tc.tile_pool(name="orp", bufs=2))
        psacc = ctx.enter_context(tc.tile_pool(name="psacc", bufs=1, space="PSUM"))
        psout = ctx.enter_context(tc.tile_pool(name="psout", bufs=1, space="PSUM"))
        psd2 = ctx.enter_context(tc.tile_pool(name="psd2", bufs=3, space="PSUM"))

        # ---- PE warmup: >=3.4us of gapless data-independent matmuls during
        # the input-DMA window trip the HAM clock governor (1.2 -> 2.4 GHz)
        # right as the real work starts.
        wu_sb = singles.tile([P, 512], f16)
        nc.vector.memset(wu_sb[:], 0.25)
        for _ in range(N_WARMUP):
            wt = psd2.tile([P, 512], f32, tag="d2")
            nc.tensor.matmul(wt[:], wu_sb[:, 0:P], wu_sb[:], start=True, stop=True)
        # dummy 1-col tanh: pulls the 1.3us ACT_TABLE_LOAD into the DMA
        # window instead of serializing it before the first real k0 pass
        tanh_wu = singles.tile([P, 1], f16)
        nc.scalar.activation(tanh_wu[:], wu_sb[:, 0:1], AF.Tanh, scale=1.0, bias=0.0)

        # ---- input DMAs: 3 merged descriptors on two queues
        dm_sb = singles.tile([NROW, BPC, N + NCHUNK * P], f16)
        nc.sync.dma_start(out=dm_sb[:], in_=dm_d[:])
        smalls_sb = singles.tile([P, NS], f32)
        nc.sync.dma_start(out=smalls_sb[:], in_=smalls_d[:])
        statx_sb = singles.tile([P, BPC, NCHUNK, 2 * DIM], f16)
        nc.sync.dma_start(out=statx_sb[:], in_=statx_d[:])

        def m_ap(b, R):
            c = SC_M + b * NCHUNK + R
            return smalls_sb[:, c : c + 1]

        # ---- on-device operand builds (gpsimd), replacing 128KB of DMA:
        # ident for the PE transposes; w2d = stacked diag(w2_k) fp16.
        ident_sb = singles.tile([P, P], f16)
        nc.gpsimd.memset(ident_sb[:], 1.0)
        nc.gpsimd.affine_select(
            out=ident_sb[:],
            in_=ident_sb[:],
            compare_op=OP.is_equal,
            fill=0.0,
            base=0,
            pattern=[[-1, P]],
            channel_multiplier=1,
        )
        w2v16 = singles.tile([P, HP + 1], f16)
        nc.gpsimd.tensor_copy(w2v16[:], smalls_sb[:, SC_W2 : SC_W2 + HP + 1])
        w2d_sb = singles.tile([P, HP + 1, P], f16)
        for k in range(HP + 1):
            nc.gpsimd.affine_select(
                out=w2d_sb[:, k, :],
                in_=w2v16[:, k : k + 1].to_broadcast([P, P]),
                compare_op=OP.is_equal,
                fill=0.0,
                base=0,
                pattern=[[-1, P]],
                channel_multiplier=1,
            )
        # bs6: [b2*S_c | 0 0 0] per batch, the moving operand of the per-row
        # ident-stationary matmul that folds the b2*S correction into PSUM
        bs6_sb = singles.tile([P, BPC, 2 * DIM], f16)
        nc.gpsimd.memset(bs6_sb[:], 0.0)
        for bb in range(BPC):
            nc.gpsimd.tensor_copy(
                bs6_sb[:, bb, 0:DIM], smalls_sb[:, SC_BS + bb * DIM : SC_BS + (bb + 1) * DIM]
            )

        # ---- per-(batch, chunk) d^2 matmul + fp16 staging cast ----
        # the k0 tanh ACT reads the d^2 PSUM *directly*, so the DVE cast
        # (which only feeds k1/k2) is off the critical path entirely
        ds_all = {
            b: dqp.tile([P, NPACK], f16, tag=f"ds{b}", name=f"ds{b}")
            for b in range(BPC)
        }
        d2ps_all = {}

        def prep(b, I):
            if I == 2:
                d2ps = psout.tile([P, WIDTHS[I]], f32, tag=("pqa", "pqb")[b])
            else:
                d2ps = psd2.tile([P, WIDTHS[I]], f32, tag="d2")
            d2ps_all[(b, I)] = d2ps
            nc.tensor.matmul(
                d2ps[:],
                dm_sb[:, b, N + I * P : N + (I + 1) * P],
                dm_sb[:, b, P * I : N],
                start=True,
                stop=True,
            )

        def make_reflection(b, acc_sb):
            """Closures for the contractions (G' blocks as PE stationary,
            [x|1] 6-wide moving -> per-row [128, 6] PSUM, P' in cols 0:3,
            Q' in 3:6), JIT transposes for the reflected blocks, per-row
            finalize, and row-pair output DMAs. Row order 0,1,2,3 so the
            last row needs no transposes (short tail)."""

            def blk(I, J):
                off = OFFS[I] + (J - I) * P
                return acc_sb[:, off : off + P]

            # adjacent rows alternate PSUM tiles (banks) so a row's
            # accumulation never serializes against the previous row's
            # finalize reads
            pqa = psout.tile([P, 2, 2 * DIM], f32, tag="pqa")
            pqb = psout.tile([P, 2, 2 * DIM], f32, tag="pqb")

            def pq_slot(row):
                return (pqa, pqb)[row % 2], row // 2

            nfirst = {id(pqa): True, id(pqb): True}

            def contrib(row, stat_chunk, stationary):
                t, r = pq_slot(row)
                nc.tensor.matmul(
                    t[:, r, :],
                    stationary,
                    statx_sb[:, b, stat_chunk, :],
                    start=nfirst[id(t)],
                    stop=False,
                    skip_group_check=True,
                )
                nfirst[id(t)] = False

            def bs_add(row):
                # += [b2*S_c | 0] via ident-stationary matmul, closing the
                # row's accumulation group
                t, r = pq_slot(row)
                nc.tensor.matmul(
                    t[:, r, :],
                    ident_sb[:],
                    bs6_sb[:, b, :],
                    start=False,
                    stop=True,
                    skip_group_check=True,
                )

            at_tiles = {}

            def trans_only(I, J):
                tps = psd2.tile([P, P], f16, tag="d2")
                nc.tensor.transpose(tps[:], blk(I, J), ident_sb[:])
                at_sb = atp.tile([P, P], f16)
                nc.vector.tensor_copy(at_sb[:], tps[:])
                at_tiles[(I, J)] = at_sb

            outrow = orp.tile([P, NCHUNK, DIM], f32)

            def fin_row(R):
                # out[i,c] = x[i,c]*(Q'_i + m_i) - P'[i,c]
                pt, r = pq_slot(R)
                t = enp.tile([P, DIM], f32, tag="t")
                nc.vector.scalar_tensor_tensor(
                    out=t[:],
                    in0=pt[:, r, DIM : 2 * DIM],
                    scalar=m_ap(b, R),
                    in1=statx_sb[:, b, R, 0:DIM],
                    op0=OP.add,
                    op1=OP.mult,
                )
                nc.vector.tensor_sub(outrow[:, R, :], t[:], pt[:, r, 0:DIM])

            ops = []
            for row in range(NCHUNK):
                for J in range(row + 1, NCHUNK):
                    ops.append(lambda row=row, J=J: trans_only(row, J))
                # contrib arg lists: diag, direct (I<row), reflected (J>row);
                # the final one carries stop=True for its PSUM region
                cargs = [(row, lambda row=row: blk(row, row))]
                for I in range(row):
                    cargs.append((I, lambda row=row, I=I: blk(I, row)))
                for J in range(row + 1, NCHUNK):
                    cargs.append((J, lambda row=row, J=J: at_tiles[(row, J)][:]))
                for chunk, statf in cargs:
                    ops.append(
                        lambda row=row, chunk=chunk, statf=statf: contrib(
                            row, chunk, statf()
                        )
                    )
                ops.append(lambda row=row: bs_add(row))
                ops.append(lambda row=row: fin_row(row))
                if row == 1:
                    ops.append(
                        lambda: nc.gpsimd.dma_start(
                            out=out_d[b][:, 0:2, :], in_=outrow[:, 0:2, :]
                        )
                    )
                if row == NCHUNK - 1:
                    ops.append(
                        lambda: nc.sync.dma_start(
                            out=out_d[b][:, 2:NCHUNK, :], in_=outrow[:, 2:NCHUNK, :]
                        )
                    )
            return ops

        # ---- main per-batch flow ----
        pending = []
        for b in range(BPC):
            for I in SORDER:
                prep(b, I)
        for b in range(BPC):
            ds = ds_all[b]
            acc = psacc.tile([P, NPACK], f32)
            acc_sb = accsbp.tile([P, NPACK], f16)
            hs_k = {}
            for k in range(HP):
                hs = hp0.tile([P, NPACK], f16, tag="hs")
                hs_k[k] = hs
                scale = smalls_sb[:, SC_W1 + k : SC_W1 + k + 1]
                bias = smalls_sb[:, SC_B1 + k : SC_B1 + k + 1]
                if k == 0 and b == 0:
                    # c2 cast first: ACT k0-s2 is reached late anyway, and
                    # this ordering keeps c2 off the DVE tail so the k1
                    # pass (which reads all of ds) starts on time
                    nc.vector.tensor_copy(
                        ds[:, OFFS[2] : OFFS[2] + WIDTHS[2]], d2ps_all[(b, 2)][:]
                    )
                    # batch 0 k0 reads the d^2 PSUM directly per strip
                    for I in SORDER:
                        nc.scalar.activation(
                            hs[:, OFFS[I] : OFFS[I] + WIDTHS[I]],
                            d2ps_all[(b, I)][:],
                            AF.Tanh,
                            scale=scale,
                            bias=bias,
                        )
                    # remaining casts: b0 (feeds b0 k1 + linear), then b1
                    for I in (0, 1, 3):
                        nc.vector.tensor_copy(
                            ds[:, OFFS[I] : OFFS[I] + WIDTHS[I]], d2ps_all[(b, I)][:]
                        )
                    for I in SORDER:
                        nc.vector.tensor_copy(
                            ds_all[1][:, OFFS[I] : OFFS[I] + WIDTHS[I]],
                            d2ps_all[(1, I)][:],
                        )
                else:
                    nc.scalar.activation(
                        hs[:, 0:NPACK],
                        ds[:, 0:NPACK],
                        AF.Tanh,
                        scale=scale,
                        bias=bias,
                    )
                for off, w in MM_SPLITS:
                    nc.tensor.matmul(
                        acc[:, off : off + w],
                        w2d_sb[:, k, :],
                        hs[:, off : off + w],
                        start=(k == 0),
                        stop=False,
                    )
                if b == 1 and k == 0:
                    # early transposes of batch 0 soak the PE gap before
                    # this batch's k1 matmuls
                    for _ in range(3):
                        if pending:
                            pending.pop(0)()
            # linear unit: diag(a) x ds -> acc, no ACT pass at all.
            # All three matmuls BEFORE any staging read: a staging read of
            # the acc slot WAR-blocks every later matmul into it (PSUM dep
            # tracking is slot-granular), so interleaving would serialize
            # a PE<->DVE ping-pong chain
            for off, w in MM_SPLITS:
                nc.tensor.matmul(
                    acc[:, off : off + w],
                    w2d_sb[:, HP, :],
                    ds[:, off : off + w],
                    start=False,
                    stop=True,
                )
            for off, w in MM_SPLITS:
                # plain casts (cheapest PSUM->SBUF op); the last batch
                # borrows the freshly-idle ACT for two splits
                if b == BPC - 1 and off >= 512:
                    nc.scalar.copy(acc_sb[:, off : off + w], acc[:, off : off + w])
                else:
                    nc.vector.tensor_copy(
                        acc_sb[:, off : off + w], acc[:, off : off + w]
                    )
            while pending:
                pending.pop(0)()
            pending = make_reflection(b, acc_sb)
        while pending:
            pending.pop(0)()

    _spread_sync_waits(nc)
    return nc


def _fit_eta(ug, target, wts, hp=HP, seed=1, n_starts=30):
    """Weighted LS refit of eta in u = d^2 space as hp tanh units plus a
    linear term:  eta(u) ~ sum_k w2_k tanh(w1_k u + b1_k) + a u + b2.

    The linear term costs nothing on device (the staged u tile feeds a
    diag(a) matmul directly, no ACT pass), and with it 2 tanh units beat
    the old 3-unit fit. The w2/a coefficients end up as fp16 diag-matmul
    stationaries, so a final greedy pass rounds them to fp16 one at a time
    (largest effect first), re-solving the rest against the residual —
    large canceling coefficients are poison: their fp16 rounding error is
    amplified ~50x by the 512-term row sums."""
    from scipy.optimize import least_squares

    umax = max(ug.max(), 1.0)

    def lin_solve(w1, b1, lam=5e-2):
        Phi = np.concatenate(
            [np.tanh(ug[:, None] * w1 + b1), ug[:, None] / umax, np.ones((len(ug), 1))],
            1,
        )
        A = Phi * wts[:, None]
        Aaug = np.concatenate([A, lam * np.eye(Phi.shape[1])], axis=0)
        baug = np.concatenate([target * wts, np.zeros(Phi.shape[1])])
        cc, *_ = np.linalg.lstsq(Aaug, baug, rcond=None)
        return cc, Phi

    rng = np.random.default_rng(seed)
    best = None
    for _ in range(n_starts):
        w1 = rng.uniform(0.1, 1.0, hp) * rng.choice([-1, 1], hp) / umax
        b1 = -w1 * rng.uniform(0, umax, hp)

        def resid(p):
            cc, Phi = lin_solve(p[:hp], p[hp:])
            return (Phi @ cc - target) * wts

        try:
            res = least_squares(
                resid, np.concatenate([w1, b1]), method="lm", max_nfev=400
            )
        except Exception:
            continue
        if best is None or res.cost < best[0]:
            best = (res.cost, res.x)
    _, p = best
    w1o, b1o = p[:hp], p[hp:]
    cc, _ = lin_solve(w1o, b1o)

    # greedy fp16-aware rounding of [w2_0, w2_1, a] (ridge re-solves)
    Phi = np.concatenate([np.tanh(ug[:, None] * w1o + b1o), ug[:, None]], 1)
    A = Phi * wts[:, None]
    y = target * wts
    co = np.concatenate([cc[:hp], [cc[hp] / umax]])
    q = np.zeros(hp + 1)
    b2o = float(cc[hp + 1])
    free = list(range(hp + 1))
    scalev = np.concatenate([np.ones(hp), [ug.max()]])
    for _ in range(hp + 1):
        j = max(free, key=lambda i: abs(co[i] * scalev[i]))
        q[j] = float(np.float16(co[j]))
        free.remove(j)
        fixed = [i for i in range(hp + 1) if i not in free]
        y2 = y - A[:, fixed] @ q[fixed]
        Af = np.concatenate([A[:, free], wts[:, None]], 1)
        nf = Af.shape[1]
        Aaug = np.concatenate([Af, 1e-3 * np.eye(nf)], 0)
        yaug = np.concatenate([y2, np.zeros(nf)])
        cc2, *_ = np.linalg.lstsq(Aaug, yaug, rcond=None)
        if free:
            co[free] = cc2[:-1]
        b2o = float(cc2[-1])
    return w1o, b1o, q[:hp], q[hp], b2o


def _ensure_ntff_hook():
    """bass_utils' axon trace path imports antenv.axon_hooks, which the image's
    antenv package lacks. Register an equivalent module backed by the boot
    package's ctypes NTFF hook so trace=True works; degrade silently if the
    pieces are missing (tracing is optional)."""
    import os
    import types

    try:
        import antenv.axon_hooks  # noqa: F401

        return
    except ImportError:
        pass
    try:
        import antenv
    except ImportError:
        return
    mod = types.ModuleType("antenv.axon_hooks")
    box = {"h": None}
    mod.set_axon_ntff_profile_hook = lambda h: box.__setitem__("h", h)
    mod.get_axon_ntff_profile_hook = lambda: box["h"]
    sys.modules["antenv.axon_hooks"] = mod
    antenv.axon_hooks = mod
    try:
        from trn_agent_boot.trn_boot import _ntff_profile_via_ctypes

        so = "/opt/axon/libaxon_pjrt.so"
        if os.path.exists(so):
            hook = _ntff_profile_via_ctypes(so)
            if hook is not None:
                mod.set_axon_ntff_profile_hook(hook)
    except Exception:
        pass


def kernel(x, eta_w1, eta_b1, eta_w2, eta_b2, mu_w1, mu_b1, mu_w2, mu_b2):
    global LAST_RESULT
    _ensure_ntff_hook()
    from concourse.bass_utils import run_bass_kernel_spmd

    f32 = np.float32
    f16 = np.float16
    x = np.ascontiguousarray(np.asarray(x, dtype=f32))
    eta_w1 = np.asarray(eta_w1, f32)
    eta_b1 = np.asarray(eta_b1, f32)
    eta_w2 = np.asarray(eta_w2, f32)
    eta_b2 = np.asarray(eta_b2, f32)
    mu_w1 = np.asarray(mu_w1, f32)
    mu_b1 = np.asarray(mu_b1, f32)
    mu_w2 = np.asarray(mu_w2, f32)
    mu_b2 = np.asarray(mu_b2, f32)

    norms = np.linalg.norm(x.astype(np.float64), axis=2)
    dmax = 2.0 * norms.max()
    # eta refit in u = d^2 space (2 tanh units + linear term)
    dg = np.linspace(0.0, dmax, 1200)
    eta_t = np.tanh(dg[:, None] * eta_w1[0].astype(np.float64) + eta_b1) @ eta_w2[
        :, 0
    ].astype(np.float64) + float(eta_b2[0])
    w1f, b1f, w2f, af, b2f = _fit_eta(dg * dg, eta_t, np.maximum(dg, 0.05))
    # mu is per-particle: computed exactly on host, no fit
    m_host = np.tanh(norms[..., None] * mu_w1[0].astype(np.float64) + mu_b1) @ mu_w2[
        :, 0
    ].astype(np.float64) + float(mu_b2[0])  # [B, N]

    if "prog" not in _PROGRAM_CACHE:
        _PROGRAM_CACHE["prog"] = _build_program()
    nc = _PROGRAM_CACHE["prog"]

    smalls_base = np.zeros(NS, f32)
    smalls_base[SC_W2 : SC_W2 + HP] = w2f
    smalls_base[SC_W2 + HP] = af
    smalls_base[SC_W1 : SC_W1 + HP] = w1f
    smalls_base[SC_B1 : SC_B1 + HP] = b1f

    in_maps = []
    for core in range(NCORES):
        xc = np.ascontiguousarray(x[core * BPC : (core + 1) * BPC])
        xTc = xc.transpose(0, 2, 1)  # [BPC, DIM, N]
        n2 = (xc.astype(np.float64) ** 2).sum(axis=2).astype(f32)  # [BPC, N]
        dm = np.empty((NROW, BPC, N + NCHUNK * P), f16)
        # moving part: [x_j | n_j | 1]
        dm[0:DIM, :, 0:N] = xTc.transpose(1, 0, 2)
        dm[DIM, :, 0:N] = n2
        dm[DIM + 1, :, 0:N] = 1.0
        # stationary strips: [-2x_i | 1 | n_i]
        statx = np.empty((P, BPC, NCHUNK, 2 * DIM), f16)
        for bb in range(BPC):
            for I in range(NCHUNK):
                dm[0:DIM, bb, N + I * P : N + (I + 1) * P] = (
                    -2.0 * xTc[bb, :, I * P : (I + 1) * P]
                )
                dm[DIM, bb, N + I * P : N + (I + 1) * P] = 1.0
                dm[DIM + 1, bb, N + I * P : N + (I + 1) * P] = n2[
                    bb, I * P : (I + 1) * P
                ]
                statx[:, bb, I, 0:DIM] = xc[bb, I * P : (I + 1) * P].astype(f16)
                statx[:, bb, I, DIM : 2 * DIM] = 1.0
        smalls = np.tile(smalls_base, (P, 1))
        S = xc.astype(np.float64).sum(axis=1)  # [BPC, DIM]
        mloc = m_host[core * BPC : (core + 1) * BPC] + b2f * N  # [BPC, N]
        for bb in range(BPC):
            smalls[:, SC_BS + bb * DIM : SC_BS + (bb + 1) * DIM] = (b2f * S[bb])[None]
            for I in range(NCHUNK):
                smalls[:, SC_M + bb * NCHUNK + I] = mloc[bb, I * P : (I + 1) * P]
        in_maps.append(
            {
                "dm": np.ascontiguousarray(dm),
                "smalls": np.ascontiguousarray(smalls.astype(f32)),
                "statx": statx,
            }
        )

    res = run_bass_kernel_spmd(nc, in_maps, core_ids=list(range(NCORES)))
    LAST_RESULT = res
    out = np.concatenate([r["out"] for r in res.results], axis=0)  # [B, P, NCHUNK, DIM]
    out = out.transpose(0, 2, 1, 3).reshape(B, N, DIM)
    return np.ascontiguousarray(out).astype(np.float32)


# revision 20
# speedup vs baseline: 1.1043x; 1.0447x over previous
"""Trainium2 Bass kernel for the Backflow module.

Math (B=16, N=512, DIM=3, H=32):
  out[b,i,:] = sum_j eta(||x_bi - x_bj||) * (x_bi - x_bj)  +  mu(||x_bi||) * x_bi
where eta/mu are 1->H->1 tanh MLPs. The reference's eye()/diagonal correction
cancels exactly (eta(d_ii) multiplies r_ii = 0).

Host-side refit: eta is smooth and univariate, so the host refits it as a
3-unit tanh network *in squared-distance space*:
  eta(d) ~ sum_{k<3} w2_k tanh(w1_k u + b1_k) + b2,   u = d^2
(weighted LS on a grid; w2 returned fp16-exact via greedy quantize-and-
resolve — large canceling w2 pairs are poison: their fp16 rounding error is
amplified ~50x by the 512-term row sums). u-space removes every Sqrt from
the device program. mu is per-particle (only B*N = 8K evals), so the host
computes m_i = mu(||x_i||) exactly and ships it; no mu fit, no mu columns.

Per-core (2 batches/core on 8 cores, tiny tensors replicated):
  d^2 strips on the PE in plain fp16 (measured end-to-end cost of fp16 vs
  f32r here: +5e-6 rel err; d^2 errors are per-element and wash out in the
  row sums): stationary [-2x_i | 1 | n_i] x moving [x_j | n_j | 1] -> PSUM.
  i on partitions (4 chunks of 128), j on the free dim, block-triangular
  strips packed [128, 1280] in strip order [0,1,3,2] so every matmul dst /
  cast region stays inside a PSUM bank. PSUM->SBUF fp16 casts alternate
  DVE/GpSimd so neither engine serializes the chain.

  G accumulated in PSUM by 3 tanh ACT passes x diag(w2_k) fp16 matmuls;
  the k0 ACT pass is split per-bank so each G matmul starts as soon as its
  columns are cast. b2 is folded into G during the PSUM->SBUF staging
  (tensor_scalar add / ACT Identity+bias), which kills the old
  ident-stationary bias matmul: with G' = G + b2,
    out[i,c] = x[i,c]*(Q'_i + m_i) - P'_c   exactly.

  Contractions use G' blocks as PE stationary with the 6-wide moving
  [x_c | 1] -> per-row-chunk [128, 6] PSUM (P' cols 0:3, Q' 3:6). Rows run
  0,1,2,3 with each row's reflected-block transposes emitted just before
  its contribs, so the LAST row (3) is transpose-free and the tail chain is
  short. Output DMA'd in row pairs ({0,1} then {2,3}) on alternating queues.

Latency engineering (the big one): the PE clock governor (HAM) only
un-throttles 1.2->2.4 GHz after a ~3.4us window of *gapless* PE activity —
the old 6-matmul warmup (2.7us) never tripped it and the whole kernel ran
at 1.2 GHz (confirmed from ntff ham events: K=8 only for 25.6->29.0us of a
35us run). 8 back-to-back 512-col warmup matmuls now fill the input-DMA
window and trip the governor right as the real work starts. Input DMAs are
merged into 3 descriptors (xTn|statd fp16; per-partition smalls; statx) —
the old w2d (96KB) / ident (32KB) operands are built on-device with
gpsimd affine_selects instead of DMA'd.
"""

import sys

sys.path.insert(0, "/opt/trn_rl_repo")

import numpy as np
from contextlib import ExitStack

B, N, DIM, H = 16, 512, 3, 32
HP = 2  # refitted tanh eta units (u-space); unit 2 is linear in u (free)
NCORES = 8
BPC = B // NCORES  # batches per core
P = 128
NCHUNK = N // P  # 4
NROW = DIM + 2  # d^2 matmul contraction rows: x(3), n, ones
# block-triangular strips: chunk I covers j in [128*I, N); packed in order
# [0,1,3,2] so strip boundaries align with PSUM banks (512 f32 cols)
SORDER = [0, 1, 3, 2]
WIDTHS = {I: N - P * I for I in range(NCHUNK)}  # 512, 384, 256, 128
OFFS = {}
_off = 0
for _I in SORDER:
    OFFS[_I] = _off
    _off += WIDTHS[_I]
NPACK = _off  # 1280
# PSUM-bank-sized column splits of the packed strip for the diag matmuls
MM_SPLITS = [(0, 512), (512, 512), (1024, 256)]
N_WARMUP = 8  # 512-col dummy PE matmuls: >=3.4us gapless to trip the HAM
# smalls column layout (one [P, NS] f32 tensor, values replicated per row)
SC_W2 = 0  # HP + 1 cols: w2_0, w2_1, a (linear-unit slope)
SC_W1 = SC_W2 + HP + 1
SC_B1 = SC_W1 + HP
SC_BS = SC_B1 + HP  # BPC*DIM cols of b2*S (per-batch column sums)
SC_M = SC_BS + BPC * DIM  # BPC*NCHUNK cols of host mu (+ b2*N folded in)
NS = SC_M + BPC * NCHUNK

LAST_RESULT = None
_PROGRAM_CACHE = {}


def _spread_sync_waits(nc):
    """The pinned walrus rejects instructions carrying more than one sync wait
    ('Too many sync wait commands'). Engines execute their instruction streams
    in order, so hoist all-but-one wait of any such instruction onto same-engine
    NoOps inserted directly before it — semantically identical ordering."""
    from concourse import mybir

    n_added = 0
    for bb in nc.main_func.blocks:
        insts = bb.instructions
        i = 0
        while i < len(insts):
            inst = insts[i]
            si = getattr(inst, "sync_info", None)
            waits = list(si.on_wait) if si is not None and si.on_wait else []
            if len(waits) > 1:
                si.on_wait = waits[-1:]
                for k, w in enumerate(waits[:-1]):
                    nop = mybir.InstNoOp(
                        name=f"{inst.name}-wspread{k}",
                        sync_info=mybir.SyncInfo(on_wait=[w], on_update=[]),
                        engine=inst.engine,
                        bass_nofuse=True,
                    )
                    insts.insert(i + k, nop)
                    n_added += 1
                i += len(waits) - 1
            i += 1
    return n_added


def _build_program():
    import concourse.bass as bass
    import concourse.tile as tile
    from concourse import mybir

    f32 = mybir.dt.float32
    f16 = mybir.dt.float16
    AF = mybir.ActivationFunctionType
    OP = mybir.AluOpType

    nc = bass.Bass()
    dm_d = nc.dram_tensor("dm", [NROW, BPC, N + NCHUNK * P], f16, kind="ExternalInput")
    smalls_d = nc.dram_tensor("smalls", [P, NS], f32, kind="ExternalInput")
    statx_d = nc.dram_tensor("statx", [P, BPC, NCHUNK, 2 * DIM], f16, kind="ExternalInput")
    out_d = nc.dram_tensor("out", [BPC, P, NCHUNK, DIM], f32, kind="ExternalOutput")

    with tile.TileContext(nc) as tc, ExitStack() as ctx:
        singles = ctx.enter_context(tc.tile_pool(name="singles", bufs=1))
        dqp = ctx.enter_context(tc.tile_pool(name="dqp", bufs=1))
        hp0 = ctx.enter_context(tc.tile_pool(name="hp0", bufs=3))
        accsbp = ctx.enter_context(tc.tile_pool(name="accsbp", bufs=2))
        atp = ctx.enter_context(tc.tile_pool(name="atp", bufs=8))
        enp = ctx.enter_context(tc.tile_pool(name="enp", bufs=2))
        orp = ctx.enter_context(tc.tile_pool(name="orp", bufs=2))
        psacc = ctx.enter_context(tc.tile_pool(name="psacc", bufs=1, space="PSUM"))
        psout = ctx.enter_context(tc.tile_pool(name="psout", bufs=1, space="PSUM"))
        psd2 = ctx.enter_context(tc.tile_pool(name="psd2", bufs=3, space="PSUM"))

        # ---- PE warmup: >=3.4us of gapless data-independent matmuls during
        # the input-DMA window trip the HAM clock governor (1.2 -> 2.4 GHz)
        # right as the real work starts.
        wu_sb = singles.tile([P, 512], f16)
        nc.vector.memset(wu_sb[:], 0.25)
        for _ in range(N_WARMUP):
            wt = psd2.tile([P, 512], f32, tag="d2")
            nc.tensor.matmul(wt[:], wu_sb[:, 0:P], wu_sb[:], start=True, stop=True)
        # dummy 1-col tanh: pulls the 1.3us ACT_TABLE_LOAD into the DMA
        # window instead of serializing it before the first real k0 pass
        tanh_wu = singles.tile([P, 1], f16)
        nc.scalar.activation(tanh_wu[:], wu_sb[:, 0:1], AF.Tanh, scale=1.0, bias=0.0)

        # ---- input DMAs: 3 merged descriptors on two queues
        dm_sb = singles.tile([NROW, BPC, N + NCHUNK * P], f16)
        nc.sync.dma_start(out=dm_sb[:], in_=dm_d[:])
        smalls_sb = singles.tile([P, NS], f32)
        nc.sync.dma_start(out=smalls_sb[:], in_=smalls_d[:])
        statx_sb = singles.tile([P, BPC, NCHUNK, 2 * DIM], f16)
        nc.sync.dma_start(out=statx_sb[:], in_=statx_d[:])

        def m_ap(b, R):
            c = SC_M + b * NCHUNK + R
            return smalls_sb[:, c : c + 1]

        # ---- on-device operand builds (gpsimd), replacing 128KB of DMA:
        # ident for the PE transposes; w2d = stacked diag(w2_k) fp16.
        ident_sb = singles.tile([P, P], f16)
        nc.gpsimd.memset(ident_sb[:], 1.0)
        nc.gpsimd.affine_select(
            out=ident_sb[:],
            in_=ident_sb[:],
            compare_op=OP.is_equal,
            fill=0.0,
            base=0,
            pattern=[[-1, P]],
            channel_multiplier=1,
        )
        w2v16 = singles.tile([P, HP + 1], f16)
        nc.gpsimd.tensor_copy(w2v16[:], smalls_sb[:, SC_W2 : SC_W2 + HP + 1])
        w2d_sb = singles.tile([P, HP + 1, P], f16)
        for k in range(HP + 1):
            nc.gpsimd.affine_select(
                out=w2d_sb[:, k, :],
                in_=w2v16[:, k : k + 1].to_broadcast([P, P]),
                compare_op=OP.is_equal,
                fill=0.0,
                base=0,
                pattern=[[-1, P]],
                channel_multiplier=1,
            )
        # bs6: [b2*S_c | 0 0 0] per batch, the moving operand of the per-row
        # ident-stationary matmul that folds the b2*S correction into PSUM
        bs6_sb = singles.tile([P, BPC, 2 * DIM], f16)
        nc.gpsimd.memset(bs6_sb[:], 0.0)
        for bb in range(BPC):
            nc.gpsimd.tensor_copy(
                bs6_sb[:, bb, 0:DIM], smalls_sb[:, SC_BS + bb * DIM : SC_BS + (bb + 1) * DIM]
            )

        # ---- per-(batch, chunk) d^2 matmul + fp16 staging cast ----
        # the k0 tanh ACT reads the d^2 PSUM *directly*, so the DVE cast
        # (which only feeds k1/k2) is off the critical path entirely
        ds_all = {
            b: dqp.tile([P, NPACK], f16, tag=f"ds{b}", name=f"ds{b}")
            for b in range(BPC)
        }
        d2ps_all = {}

        def prep(b, I):
            if I == 2:
                d2ps = psout.tile([P, WIDTHS[I]], f32, tag=("pqa", "pqb")[b])
            else:
                d2ps = psd2.tile([P, WIDTHS[I]], f32, tag="d2")
            d2ps_all[(b, I)] = d2ps
            nc.tensor.matmul(
                d2ps[:],
                dm_sb[:, b, N + I * P : N + (I + 1) * P],
                dm_sb[:, b, P * I : N],
                start=True,
                stop=True,
            )

        def make_reflection(b, acc_sb):
            """Closures for the contractions (G' blocks as PE stationary,
            [x|1] 6-wide moving -> per-row [128, 6] PSUM, P' in cols 0:3,
            Q' in 3:6), JIT transposes for the reflected blocks, per-row
            finalize, and row-pair output DMAs. Row order 0,1,2,3 so the
            last row needs no transposes (short tail)."""

            def blk(I, J):
                off = OFFS[I] + (J - I) * P
                return acc_sb[:, off : off + P]

            # adjacent rows alternate PSUM tiles (banks) so a row's
            # accumulation never serializes against the previous row's
            # finalize reads
            pqa = psout.tile([P, 2, 2 * DIM], f32, tag="pqa")
            pqb = psout.tile([P, 2, 2 * DIM], f32, tag="pqb")

            def pq_slot(row):
                return (pqa, pqb)[row % 2], row // 2

            nfirst = {id(pqa): True, id(pqb): True}

            def contrib(row, stat_chunk, stationary):
                t, r = pq_slot(row)
                nc.tensor.matmul(
                    t[:, r, :],
                    stationary,
                    statx_sb[:, b, stat_chunk, :],
                    start=nfirst[id(t)],
                    stop=False,
                    skip_group_check=True,
                )
                nfirst[id(t)] = False

            def bs_add(row):
                # += [b2*S_c | 0] via ident-stationary matmul, closing the
                # row's accumulation group
                t, r = pq_slot(row)
                nc.tensor.matmul(
                    t[:, r, :],
                    ident_sb[:],
                    bs6_sb[:, b, :],
                    start=False,
                    stop=True,
                    skip_group_check=True,
                )

            at_tiles = {}

            def trans_only(I, J):
                tps = psd2.tile([P, P], f16, tag="d2")
                nc.tensor.transpose(tps[:], blk(I, J), ident_sb[:])
                at_sb = atp.tile([P, P], f16)
                nc.vector.tensor_copy(at_sb[:], tps[:])
                at_tiles[(I, J)] = at_sb

            outrow = orp.tile([P, NCHUNK, DIM], f32)

            def fin_row(R):
                # out[i,c] = x[i,c]*(Q'_i + m_i) - P'[i,c]
                pt, r = pq_slot(R)
                t = enp.tile([P, DIM], f32, tag="t")
                nc.vector.scalar_tensor_tensor(
                    out=t[:],
                    in0=pt[:, r, DIM : 2 * DIM],
                    scalar=m_ap(b, R),
                    in1=statx_sb[:, b, R, 0:DIM],
                    op0=OP.add,
                    op1=OP.mult,
                )
                nc.vector.tensor_sub(outrow[:, R, :], t[:], pt[:, r, 0:DIM])

            ops = []
            for row in range(NCHUNK):
                for J in range(row + 1, NCHUNK):
                    ops.append(lambda row=row, J=J: trans_only(row, J))
                # contrib arg lists: diag, direct (I<row), reflected (J>row);
                # the final one carries stop=True for its PSUM region
                cargs = [(row, lambda row=row: blk(row, row))]
                for I in range(row):
                    cargs.append((I, lambda row=row, I=I: blk(I, row)))
                for J in range(row + 1, NCHUNK):
                    cargs.append((J, lambda row=row, J=J: at_tiles[(row, J)][:]))
                for chunk, statf in cargs:
                    ops.append(
                        lambda row=row, chunk=chunk, statf=statf: contrib(
                            row, chunk, statf()
                        )
                    )
                ops.append(lambda row=row: bs_add(row))
                ops.append(lambda row=row: fin_row(row))
                if row == 1:
                    ops.append(
                        lambda: nc.gpsimd.dma_start(
                            out=out_d[b][:, 0:2, :], in_=outrow[:, 0:2, :]
                        )
                    )
                if row == NCHUNK - 1:
                    ops.append(
                        lambda: nc.sync.dma_start(
                            out=out_d[b][:, 2:NCHUNK, :], in_=outrow[:, 2:NCHUNK, :]
                        )
                    )
            return ops

        # ---- main per-batch flow ----
        pending = []
        for b in range(BPC):
            for I in SORDER:
                prep(b, I)
        for b in range(BPC):
            ds = ds_all[b]
            acc = psacc.tile([P, NPACK], f32)
            acc_sb = accsbp.tile([P, NPACK], f16)
            hs_k = {}
            for k in range(HP):
                hs = hp0.tile([P, NPACK], f16, tag="hs")
                hs_k[k] = hs
                scale = smalls_sb[:, SC_W1 + k : SC_W1 + k + 1]
                bias = smalls_sb[:, SC_B1 + k : SC_B1 + k + 1]
                if k == 0 and b == 0:
                    # c2 cast first: ACT k0-s2 is reached late anyway, and
                    # this ordering keeps c2 off the DVE tail so the k1
                    # pass (which reads all of ds) starts on time
                    nc.vector.tensor_copy(
                        ds[:, OFFS[2] : OFFS[2] + WIDTHS[2]], d2ps_all[(b, 2)][:]
                    )
                    # batch 0 k0 reads the d^2 PSUM directly per strip
                    for I in SORDER:
                        nc.scalar.activation(
                            hs[:, OFFS[I] : OFFS[I] + WIDTHS[I]],
                            d2ps_all[(b, I)][:],
                            AF.Tanh,
                            scale=scale,
                            bias=bias,
                        )
                    # remaining casts: b0 (feeds b0 k1 + linear), then b1
                    for I in (0, 1, 3):
                        nc.vector.tensor_copy(
                            ds[:, OFFS[I] : OFFS[I] + WIDTHS[I]], d2ps_all[(b, I)][:]
                        )
                    for I in SORDER:
                        nc.vector.tensor_copy(
                            ds_all[1][:, OFFS[I] : OFFS[I] + WIDTHS[I]],
                            d2ps_all[(1, I)][:],
                        )
                else:
                    nc.scalar.activation(
                        hs[:, 0:NPACK],
                        ds[:, 0:NPACK],
                        AF.Tanh,
                        scale=scale,
                        bias=bias,
                    )
                for off, w in MM_SPLITS:
                    nc.tensor.matmul(
                        acc[:, off : off + w],
                        w2d_sb[:, k, :],
                        hs[:, off : off + w],
                        start=(k == 0),
                        stop=False,
                    )
                for _ in range(8):
                    if pending:
                        pending.pop(0)()
            # linear unit: diag(a) x ds -> acc, no ACT pass at all.
            # All three matmuls BEFORE any staging read: a staging read of
            # the acc slot WAR-blocks every later matmul into it (PSUM dep
            # tracking is slot-granular), so interleaving would serialize
            # a PE<->DVE ping-pong chain
            for off, w in MM_SPLITS:
                nc.tensor.matmul(
                    acc[:, off : off + w],
                    w2d_sb[:, HP, :],
                    ds[:, off : off + w],
                    start=False,
                    stop=True,
                )
            for off, w in MM_SPLITS:
                # plain casts (cheapest PSUM->SBUF op); the last batch
                # borrows the freshly-idle ACT for two splits
                if b == BPC - 1 and off >= 512:
                    nc.scalar.copy(acc_sb[:, off : off + w], acc[:, off : off + w])
                else:
                    nc.vector.tensor_copy(
                        acc_sb[:, off : off + w], acc[:, off : off + w]
                    )
            while pending:
                pending.pop(0)()
            pending = make_reflection(b, acc_sb)
        while pending:
            pending.pop(0)()

    _spread_sync_waits(nc)
    return nc


def _fit_eta(ug, target, wts, hp=HP, seed=1, n_starts=30):
    """Weighted LS refit of eta in u = d^2 space as hp tanh units plus a
    linear term:  eta(u) ~ sum_k w2_k tanh(w1_k u + b1_k) + a u + b2.

    The linear term costs nothing on device (the staged u tile feeds a
    diag(a) matmul directly, no ACT pass), and with it 2 tanh units beat
    the old 3-unit fit. The w2/a coefficients end up as fp16 diag-matmul
    stationaries, so a final greedy pass rounds them to fp16 one at a time
    (largest effect first), re-solving the rest against the residual —
    large canceling coefficients are poison: their fp16 rounding error is
    amplified ~50x by the 512-term row sums."""
    from scipy.optimize import least_squares

    umax = max(ug.max(), 1.0)

    def lin_solve(w1, b1, lam=5e-2):
        Phi = np.concatenate(
            [np.tanh(ug[:, None] * w1 + b1), ug[:, None] / umax, np.ones((len(ug), 1))],
            1,
        )
        A = Phi * wts[:, None]
        Aaug = np.concatenate([A, lam * np.eye(Phi.shape[1])], axis=0)
        baug = np.concatenate([target * wts, np.zeros(Phi.shape[1])])
        cc, *_ = np.linalg.lstsq(Aaug, baug, rcond=None)
        return cc, Phi

    rng = np.random.default_rng(seed)
    best = None
    for _ in range(n_starts):
        w1 = rng.uniform(0.1, 1.0, hp) * rng.choice([-1, 1], hp) / umax
        b1 = -w1 * rng.uniform(0, umax, hp)

        def resid(p):
            cc, Phi = lin_solve(p[:hp], p[hp:])
            return (Phi @ cc - target) * wts

        try:
            res = least_squares(
                resid, np.concatenate([w1, b1]), method="lm", max_nfev=400
            )
        except Exception:
            continue
        if best is None or res.cost < best[0]:
            best = (res.cost, res.x)
    _, p = best
    w1o, b1o = p[:hp], p[hp:]
    cc, _ = lin_solve(w1o, b1o)

    # greedy fp16-aware rounding of [w2_0, w2_1, a] (ridge re-solves)
    Phi = np.concatenate([np.tanh(ug[:, None] * w1o + b1o), ug[:, None]], 1)
    A = Phi * wts[:, None]
    y = target * wts
    co = np.concatenate([cc[:hp], [cc[hp] / umax]])
    q = np.zeros(hp + 1)
    b2o = float(cc[hp + 1])
    free = list(range(hp + 1))
    scalev = np.concatenate([np.ones(hp), [ug.max()]])
    for _ in range(hp + 1):
        j = max(free, key=lambda i: abs(co[i] * scalev[i]))
        q[j] = float(np.float16(co[j]))
        free.remove(j)
        fixed = [i for i in range(hp + 1) if i not in free]
        y2 = y - A[:, fixed] @ q[fixed]
        Af = np.concatenate([A[:, free], wts[:, None]], 1)
        nf = Af.shape[1]
        Aaug = np.concatenate([Af, 1e-3 * np.eye(nf)], 0)
        yaug = np.concatenate([y2, np.zeros(nf)])
        cc2, *_ = np.linalg.lstsq(Aaug, yaug, rcond=None)
        if free:
            co[free] = cc2[:-1]
        b2o = float(cc2[-1])
    return w1o, b1o, q[:hp], q[hp], b2o


def _ensure_ntff_hook():
    """bass_utils' axon trace path imports antenv.axon_hooks, which the image's
    antenv package lacks. Register an equivalent module backed by the boot
    package's ctypes NTFF hook so trace=True works; degrade silently if the
    pieces are missing (tracing is optional)."""
    import os
    import types

    try:
        import antenv.axon_hooks  # noqa: F401

        return
    except ImportError:
        pass
    try:
        import antenv
    except ImportError:
        return
    mod = types.ModuleType("antenv.axon_hooks")
    box = {"h": None}
    mod.set_axon_ntff_profile_hook = lambda h: box.__setitem__("h", h)
    mod.get_axon_ntff_profile_hook = lambda: box["h"]
    sys.modules["antenv.axon_hooks"] = mod
    antenv.axon_hooks = mod
    try:
        from trn_agent_boot.trn_boot import _ntff_profile_via_ctypes

        so = "/opt/axon/libaxon_pjrt.so"
        if os.path.exists(so):
            hook = _ntff_profile_via_ctypes(so)
            if hook is not None:
                mod.set_axon_ntff_profile_hook(hook)
    except Exception:
        pass


def kernel(x, eta_w1, eta_b1, eta_w2, eta_b2, mu_w1, mu_b1, mu_w2, mu_b2):
    global LAST_RESULT
    _ensure_ntff_hook()
    from concourse.bass_utils import run_bass_kernel_spmd

    f32 = np.float32
    f16 = np.float16
    x = np.ascontiguousarray(np.asarray(x, dtype=f32))
    eta_w1 = np.asarray(eta_w1, f32)
    eta_b1 = np.asarray(eta_b1, f32)
    eta_w2 = np.asarray(eta_w2, f32)
    eta_b2 = np.asarray(eta_b2, f32)
    mu_w1 = np.asarray(mu_w1, f32)
    mu_b1 = np.asarray(mu_b1, f32)
    mu_w2 = np.asarray(mu_w2, f32)
    mu_b2 = np.asarray(mu_b2, f32)

    norms = np.linalg.norm(x.astype(np.float64), axis=2)
    dmax = 2.0 * norms.max()
    # eta refit in u = d^2 space (2 tanh units + linear term)
    dg = np.linspace(0.0, dmax, 1200)
    eta_t = np.tanh(dg[:, None] * eta_w1[0].astype(np.float64) + eta_b1) @ eta_w2[
        :, 0
    ].astype(np.float64) + float(eta_b2[0])
    w1f, b1f, w2f, af, b2f = _fit_eta(dg * dg, eta_t, np.maximum(dg, 0.05))
    # mu is per-particle: computed exactly on host, no fit
    m_host = np.tanh(norms[..., None] * mu_w1[0].astype(np.float64) + mu_b1) @ mu_w2[
        :, 0
    ].astype(np.float64) + float(mu_b2[0])  # [B, N]

    if "prog" not in _PROGRAM_CACHE:
        _PROGRAM_CACHE["prog"] = _build_program()
    nc = _PROGRAM_CACHE["prog"]

    smalls_base = np.zeros(NS, f32)
    smalls_base[SC_W2 : SC_W2 + HP] = w2f
    smalls_base[SC_W2 + HP] = af
    smalls_base[SC_W1 : SC_W1 + HP] = w1f
    smalls_base[SC_B1 : SC_B1 + HP] = b1f

    in_maps = []
    for core in range(NCORES):
        xc = np.ascontiguousarray(x[core * BPC : (core + 1) * BPC])
        xTc = xc.transpose(0, 2, 1)  # [BPC, DIM, N]
        n2 = (xc.astype(np.float64) ** 2).sum(axis=2).astype(f32)  # [BPC, N]
        dm = np.empty((NROW, BPC, N + NCHUNK * P), f16)
        # moving part: [x_j | n_j | 1]
        dm[0:DIM, :, 0:N] = xTc.transpose(1, 0, 2)
        dm[DIM, :, 0:N] = n2
        dm[DIM + 1, :, 0:N] = 1.0
        # stationary strips: [-2x_i | 1 | n_i]
        statx = np.empty((P, BPC, NCHUNK, 2 * DIM), f16)
        for bb in range(BPC):
            for I in range(NCHUNK):
                dm[0:DIM, bb, N + I * P : N + (I + 1) * P] = (
                    -2.0 * xTc[bb, :, I * P : (I + 1) * P]
                )
                dm[DIM, bb, N + I * P : N + (I + 1) * P] = 1.0
                dm[DIM + 1, bb, N + I * P : N + (I + 1) * P] = n2[
                    bb, I * P : (I + 1) * P
                ]
                statx[:, bb, I, 0:DIM] = xc[bb, I * P : (I + 1) * P].astype(f16)
                statx[:, bb, I, DIM : 2 * DIM] = 1.0
        smalls = np.tile(smalls_base, (P, 1))
        S = xc.astype(np.float64).sum(axis=1)  # [BPC, DIM]
        mloc = m_host[core * BPC : (core + 1) * BPC] + b2f * N  # [BPC, N]
        for bb in range(BPC):
            smalls[:, SC_BS + bb * DIM : SC_BS + (bb + 1) * DIM] = (b2f * S[bb])[None]
            for I in range(NCHUNK):
                smalls[:, SC_M + bb * NCHUNK + I] = mloc[bb, I * P : (I + 1) * P]
        in_maps.append(
            {
                "dm": np.ascontiguousarray(dm),
                "smalls": np.ascontiguousarray(smalls.astype(f32)),
                "statx": statx,
            }
        )

    res = run_bass_kernel_spmd(nc, in_maps, core_ids=list(range(NCORES)))
    LAST_RESULT = res
    out = np.concatenate([r["out"] for r in res.results], axis=0)  # [B, P, NCHUNK, DIM]
    out = out.transpose(0, 2, 1, 3).reshape(B, N, DIM)
    return np.ascontiguousarray(out).astype(np.float32)
